# revision 10
# baseline (speedup 1.0000x reference)
"""Trainium2 Bass kernel for the DTGL GCN+windowed-LSTM module.

Computation (see reference):
  h = relu(adj @ (x @ Wg0 + bg0));  h = relu(adj @ (h @ Wg1 + bg1))
  for p in 1..4: run LSTM_p over disjoint length-p windows of h (zero init
  state), writing the last hidden state back at each window end (in place).

Sharding: pure data-parallel over batch B=64 across 8 cores (8 batches per
core); adj and all weights replicated. No collectives.

Per-core design (all matmul inputs bf16, fp32 PSUM accumulation; LSTM cell
state kept fp32 for accuracy):
  - GCN bias reassociated: adj @ (x@W + 1 b^T) = (adj@x)@W + rowsum(adj)(x)b,
    so every matmul keeps its contraction dim on partitions:
      1A: z1T[d,u] = sum_t x[t,d]*adjT[t,u]
      1B: h1[u,h]  = relu(sum_d z1T[d,u]*Wg0[d,h] + rs[u]*bg0[h])
      2A: z2T[h,u] = sum_t h1[t,h]*adjT[t,u]
      2B: h2T[h,u] = relu(sum_k Wg1[k,h]*z2T[k,u] + bg1[h]*rs[u])
  - h2T stays feature-major bf16 in SBUF; the 4 LSTM passes update it in
    place (gates via PSUM-accumulated bf16 matmuls, sigmoid/tanh on ACT with
    fused per-partition bias, cell math on DVE, window gathers on gpsimd).
  - Output is DMA'd out feature-major bf16; the host transposes/casts.

Batches run in 4 groups of 2; group g+1's GCN matmul blocks are interleaved
between group g's LSTM unit-pairs so the tensor engine stays busy while the
(scalar-engine-bound) LSTM activations drain.  LSTM chunks are processed in
pairs (A/B) with one-gate-at-a-time PSUM rotation (2 tags x 2 banks) so the
recurrent dependency never stalls the PE.
"""

import numpy as np

B, T, D, H = 64, 2048, 256, 256
MAX_SKIP = 4
NCORES = 8
BL = B // NCORES          # batches per core
GJ = 2                    # batches per group
NGRP = BL // GJ           # 4 groups
TK = T // 128             # 16 t-chunks
UC = T // 512             # 4 u-chunks of 512
HK = H // 128             # 2 feature blocks
JB0 = {"i": 0, "f": 2, "g": 4, "o": 6}
# window chunking per pass: list of (ws, cw)
CHUNKS = {
    1: [(0, 512), (512, 512), (1024, 512), (1536, 512)],
    2: [(0, 512), (512, 512)],
    3: [(0, 341), (341, 341)],
    4: [(0, 512)],
}

_COMPILED = None


def _build_program():
    import concourse.mybir as mybir
    import concourse.tile as tile
    from concourse import bacc

    f32 = mybir.dt.float32
    bf16 = mybir.dt.bfloat16

    nc = bacc.Bacc("TRN2", target_bir_lowering=False, debug=False)

    io = dict(
        x=nc.dram_tensor("x", [BL, T, D], bf16, kind="ExternalInput").ap(),
        adjT=nc.dram_tensor("adjT", [T, T], bf16, kind="ExternalInput").ap(),
        rs=nc.dram_tensor("rs", [1, T], bf16, kind="ExternalInput").ap(),
        wg0=nc.dram_tensor("wg0", [D, H], bf16, kind="ExternalInput").ap(),
        wg1=nc.dram_tensor("wg1", [D, H], bf16, kind="ExternalInput").ap(),
        bg0=nc.dram_tensor("bg0", [1, H], bf16, kind="ExternalInput").ap(),
        bg1=nc.dram_tensor("bg1", [1, H], bf16, kind="ExternalInput").ap(),
        wihT=nc.dram_tensor("wihT", [MAX_SKIP, H, 4 * H], bf16, kind="ExternalInput").ap(),
        whhT=nc.dram_tensor("whhT", [MAX_SKIP, H, 4 * H], bf16, kind="ExternalInput").ap(),
        biasT=nc.dram_tensor("biasT", [128, MAX_SKIP * 8], f32, kind="ExternalInput").ap(),
        out=nc.dram_tensor("out", [BL, HK, 128, T], bf16, kind="ExternalOutput").ap(),
    )

    with tile.TileContext(nc) as tc:
        _emit(nc, tc, mybir, io)

    nc.compile()
    return nc


class _Ctx:
    pass


def _emit(nc, tc, mybir, io):
    from contextlib import ExitStack

    f32 = mybir.dt.float32
    bf16 = mybir.dt.bfloat16
    AF = mybir.ActivationFunctionType

    c = _Ctx()
    c.nc, c.io, c.f32, c.bf16, c.AF = nc, io, f32, bf16, AF

    with ExitStack() as root:
        # ---- constants / weights ----
        cp = root.enter_context(tc.tile_pool(name="const", bufs=1))
        c.wg0_sb = cp.tile([128, HK * H], bf16, name="wg0_sb")
        c.wg1_sb = cp.tile([128, HK * H], bf16, name="wg1_sb")
        for hk in range(HK):
            nc.scalar.dma_start(out=c.wg0_sb[:, hk * H:(hk + 1) * H],
                                in_=io["wg0"][hk * 128:(hk + 1) * 128, :])
            nc.scalar.dma_start(out=c.wg1_sb[:, hk * H:(hk + 1) * H],
                                in_=io["wg1"][hk * 128:(hk + 1) * 128, :])
        c.bg0_sb = cp.tile([1, H], bf16, name="bg0_sb")
        c.bg1_sb = cp.tile([1, H], bf16, name="bg1_sb")
        c.rs_sb = cp.tile([1, T], bf16, name="rs_sb")
        c.biasT_sb = cp.tile([128, MAX_SKIP * 8], f32, name="biasT_sb")
        nc.scalar.dma_start(out=c.bg0_sb[:], in_=io["bg0"][:])
        nc.scalar.dma_start(out=c.bg1_sb[:], in_=io["bg1"][:])
        nc.scalar.dma_start(out=c.rs_sb[:], in_=io["rs"][:])
        nc.scalar.dma_start(out=c.biasT_sb[:], in_=io["biasT"][:])

        # ---- persistent pools ----
        c.h2t_pool = root.enter_context(tc.tile_pool(name="h2tp", bufs=1))
        c.h1_pool = root.enter_context(tc.tile_pool(name="h1p", bufs=1))
        c.x_pool = root.enter_context(tc.tile_pool(name="xp", bufs=1))
        c.adjt_pool = root.enter_context(tc.tile_pool(name="adjt", bufs=2))
        c.z_pool = root.enter_context(tc.tile_pool(name="zp", bufs=2))
        c.zps = root.enter_context(tc.tile_pool(name="zps", bufs=1, space="PSUM"))
        c.mps = root.enter_context(tc.tile_pool(name="mps", bufs=2, space="PSUM"))
        c.gps = root.enter_context(tc.tile_pool(name="gps", bufs=1, space="PSUM"))
        c.act_pool = root.enter_context(tc.tile_pool(name="actp", bufs=1))
        c.st_pool = root.enter_context(tc.tile_pool(name="stp", bufs=1))
        c.xc_pool = root.enter_context(tc.tile_pool(name="xcp", bufs=1))

        # LSTM weights resident (loaded up front; consumed much later)
        lwp = root.enter_context(tc.tile_pool(name="lw", bufs=1))
        c.wih = {}
        c.whh = {}
        for p in range(1, MAX_SKIP + 1):
            w = lwp.tile([128, HK * 4 * H], bf16, name=f"wih{p}")
            for hk in range(HK):
                nc.scalar.dma_start(out=w[:, hk * 4 * H:(hk + 1) * 4 * H],
                                    in_=io["wihT"][p - 1, hk * 128:(hk + 1) * 128, :])
            c.wih[p] = w
            if p > 1:
                w2 = lwp.tile([128, HK * 4 * H], bf16, name=f"whh{p}")
                for hk in range(HK):
                    nc.scalar.dma_start(out=w2[:, hk * 4 * H:(hk + 1) * 4 * H],
                                        in_=io["whhT"][p - 1, hk * 128:(hk + 1) * 128, :])
                c.whh[p] = w2

        # h2t slabs: 4 tags (2 groups in flight); h1: 2 tags; x: 2x16 tags
        c.h2t = {}   # batch index -> tile (assigned per group)
        c.qidx = 0   # rotating LSTM gate-psum tag

        # group 0 GCN emitted densely
        for blk in _gcn_blocks(c, tc, 0):
            blk()
        # interleave: LSTM(g) with GCN(g+1)
        for g in range(NGRP):
            gcn_next = _gcn_blocks(c, tc, g + 1) if g + 1 < NGRP else []
            gi = 0
            for pair in _lstm_pairs(c, tc, g):
                pair()
                for _ in range(2):
                    if gi < len(gcn_next):
                        gcn_next[gi]()
                        gi += 1
            while gi < len(gcn_next):
                gcn_next[gi]()
                gi += 1
            # output DMA for this group's batches
            for jj in range(GJ):
                b = g * GJ + jj
                slab = c.h2t[b]
                for hk in range(HK):
                    nc.gpsimd.dma_start(out=io["out"][b, hk],
                                        in_=slab[:, hk * T:(hk + 1) * T])


def _gcn_blocks(c, tc, g):
    """Return list of emission callbacks for group g's GCN (both layers)."""
    if g >= NGRP:
        return []
    nc, io, f32, bf16, AF = c.nc, c.io, c.f32, c.bf16, c.AF
    blocks = []

    xs = {}
    h1 = {}

    def load_x():
        for jj in range(GJ):
            b = g * GJ + jj
            h1[jj] = c.h1_pool.tile([128, TK * H], bf16, name=f"h1_{jj}",
                                    tag=f"h1_{jj}")
            for k in range(TK):
                xt = c.x_pool.tile([128, D], bf16, name="xt", tag=f"x_{jj}_{k}")
                nc.sync.dma_start(out=xt[:], in_=io["x"][b, k * 128:(k + 1) * 128, :])
                xs[(jj, k)] = xt
            c.h2t[b] = c.h2t_pool.tile([128, HK * T], bf16, name=f"h2t_{b % 4}",
                                       tag=f"h2t_{b % 4}")
    blocks.append(load_x)

    adjt = {}

    def load_adjt(u4):
        us = u4 * 512
        for k in range(TK):
            a = c.adjt_pool.tile([128, 512], bf16, name=f"adjt_{k}", tag=f"a_{k}")
            nc.sync.dma_start(out=a[:], in_=io["adjT"][k * 128:(k + 1) * 128, us:us + 512])
            adjt[k] = a

    def ph1_block(u4, jj):
        if jj == 0:
            load_adjt(u4)
        us = u4 * 512
        zp = c.zps.tile([128, 1024], f32, name="zp", tag="zp")
        for k in range(TK):
            for dk in range(HK):
                nc.tensor.matmul(zp[:, dk * 512:(dk + 1) * 512],
                                 xs[(jj, k)][:, dk * 128:(dk + 1) * 128],
                                 adjt[k][:],
                                 start=(k == 0), stop=(k == TK - 1))
        z1t = c.z_pool.tile([128, 1024], bf16, name="z1t", tag="z1t")
        nc.vector.tensor_copy(z1t[:], zp[:])
        for ub in range(4):
            ua = us + ub * 128
            hp = c.mps.tile([128, 512], f32, name="hp", tag="mp")
            for dk in range(HK):
                nc.tensor.matmul(hp[:, :H],
                                 z1t[:, dk * 512 + ub * 128: dk * 512 + (ub + 1) * 128],
                                 c.wg0_sb[:, dk * H:(dk + 1) * H],
                                 start=(dk == 0), stop=False)
            nc.tensor.matmul(hp[:, :H], c.rs_sb[0:1, ua:ua + 128], c.bg0_sb[0:1, :],
                             start=False, stop=True)
            nc.vector.tensor_relu(h1[jj][:, (u4 * 4 + ub) * H:(u4 * 4 + ub + 1) * H],
                                  hp[:, :H])

    def ph2_block(u4, jj):
        if jj == 0:
            load_adjt(u4)
        us = u4 * 512
        b = g * GJ + jj
        slab = c.h2t[b]
        zp = c.zps.tile([128, 1024], f32, name="zp2", tag="zp")
        for hk in range(HK):
            for k in range(TK):
                nc.tensor.matmul(zp[:, hk * 512:(hk + 1) * 512],
                                 h1[jj][:, k * H + hk * 128: k * H + (hk + 1) * 128],
                                 adjt[k][:],
                                 start=(k == 0), stop=(k == TK - 1))
        z2t = c.z_pool.tile([128, 1024], bf16, name="z2t", tag="z2t")
        nc.vector.tensor_copy(z2t[:], zp[:])
        for ho in range(HK):
            hp = c.mps.tile([128, 512], f32, name="hp2", tag="mp")
            for hk in range(HK):
                nc.tensor.matmul(hp[:],
                                 c.wg1_sb[:, hk * H + ho * 128: hk * H + (ho + 1) * 128],
                                 z2t[:, hk * 512:(hk + 1) * 512],
                                 start=(hk == 0), stop=False)
            nc.tensor.matmul(hp[:], c.bg1_sb[0:1, ho * 128:(ho + 1) * 128],
                             c.rs_sb[0:1, us:us + 512], start=False, stop=True)
            nc.vector.tensor_relu(slab[:, ho * T + us: ho * T + us + 512], hp[:])

    for u4 in range(UC):
        for jj in range(GJ):
            blocks.append(lambda u4=u4, jj=jj: ph1_block(u4, jj))
    for u4 in range(UC):
        for jj in range(GJ):
            blocks.append(lambda u4=u4, jj=jj: ph2_block(u4, jj))
    return blocks


def _lstm_pairs(c, tc, g):
    """Yield emission callbacks, one per unit-pair, passes p=1..4 in order."""
    for p in range(1, MAX_SKIP + 1):
        units = [(jj, ws, cw) for jj in range(GJ) for (ws, cw) in CHUNKS[p]]
        for i in range(0, len(units), 2):
            pair = units[i:i + 2]
            yield lambda p=p, pair=tuple(pair): _emit_pair(c, tc, g, p, pair)


def _emit_pair(c, tc, g, p, pair):
    nc, f32, bf16, AF = c.nc, c.f32, c.bf16, c.AF
    nw = T // p

    # per-unit state dicts
    st = []
    for slot, (jj, ws, cw) in enumerate(pair):
        b = g * GJ + jj
        slab = c.h2t[b]
        view = [slab[:, hk * T: hk * T + nw * p].rearrange("a (w q) -> a w q", q=p)
                for hk in range(HK)]
        u = dict(slot=slot, jj=jj, ws=ws, cw=cw, view=view, slab=slab,
                 c_t=None, h_t=None, xc={})
        st.append(u)

    # prefetch gathers for all steps (gpsimd); pass>1 only
    if p > 1:
        for u in st:
            for t in range(p):
                xc = c.xc_pool.tile([128, 1024], bf16, name="xc",
                                    tag=f"xc{u['slot']}_{t}")
                for hk in range(HK):
                    nc.gpsimd.tensor_copy(
                        xc[:, hk * 512: hk * 512 + u["cw"]],
                        u["view"][hk][:, u["ws"]:u["ws"] + u["cw"], t:t + 1])
                u["xc"][t] = xc

    for t in range(p):
        gates = "igo" if t == 0 else "ifgo"
        for u in st:
            cw, ws, slot = u["cw"], u["ws"], u["slot"]
            n2 = [slice(0, cw), slice(512, 512 + cw)]
            act = {}
            for gn in gates:
                # one gate = one psum quarter (rotating tag)
                q = c.gps.tile([128, 1024], f32, name="q", tag=f"q{c.qidx % 2}")
                c.qidx += 1
                for half in range(2):
                    jb = JB0[gn] + half
                    o = q[:, half * 512: half * 512 + cw]
                    for hk in range(HK):
                        if p == 1:
                            rhs = u["slab"][:, hk * T + ws: hk * T + ws + cw]
                        else:
                            rhs = u["xc"][t][:, hk * 512: hk * 512 + cw]
                        nc.tensor.matmul(
                            o,
                            c.wih[p][:, hk * 4 * H + jb * 128: hk * 4 * H + (jb + 1) * 128],
                            rhs,
                            start=(hk == 0),
                            stop=(t == 0 and hk == HK - 1))
                    if t > 0:
                        for hk in range(HK):
                            nc.tensor.matmul(
                                o,
                                c.whh[p][:, hk * 4 * H + jb * 128: hk * 4 * H + (jb + 1) * 128],
                                u["h_t"][:, hk * 512: hk * 512 + cw],
                                start=False, stop=(hk == HK - 1))
                # activations straight after this gate's matmuls
                fn = AF.Tanh if gn == "g" else AF.Sigmoid
                a = c.act_pool.tile([128, 1024], bf16, name=f"a_{gn}",
                                    tag=f"a{slot}_{gn}")
                act[gn] = a
                for half in range(2):
                    col = (p - 1) * 8 + JB0[gn] + half
                    nc.scalar.activation(
                        a[:, half * 512: half * 512 + cw],
                        q[:, half * 512: half * 512 + cw],
                        fn, bias=c.biasT_sb[:, col:col + 1])

            # cell math (DVE); strided [128, 2, cw] views over the 1024 tiles
            def v2(tile):
                return tile.rearrange("a (h w) -> a h w", h=2)[:, :, :cw]
            if t == 0:
                cn = c.st_pool.tile([128, 1024], f32, name="cn", tag=f"c{slot}")
                nc.vector.tensor_mul(v2(cn), v2(act["i"]), v2(act["g"]))
                u["c_t"] = cn
            else:
                ig = c.st_pool.tile([128, 1024], bf16, name="ig", tag=f"ig{slot}")
                nc.vector.tensor_mul(v2(ig), v2(act["i"]), v2(act["g"]))
                cn = u["c_t"]
                nc.vector.tensor_mul(v2(cn), v2(act["f"]), v2(cn))
                nc.vector.tensor_add(v2(cn), v2(cn), v2(ig))
            # tanh(c) on ACT (single strided instr, no bias)
            tc_t = c.st_pool.tile([128, 1024], bf16, name="tc", tag=f"tc{slot}",
                                  bufs=2)
            nc.scalar.activation(v2(tc_t), v2(u["c_t"]), AF.Tanh)
            if t == p - 1:
                for hk in range(HK):
                    nc.vector.tensor_mul(
                        u["view"][hk][:, ws:ws + cw, p - 1:p],
                        act["o"][:, hk * 512: hk * 512 + cw],
                        tc_t[:, hk * 512: hk * 512 + cw])
            else:
                hn = c.st_pool.tile([128, 1024], bf16, name="hn", tag=f"h{slot}",
                                    bufs=2)
                nc.vector.tensor_mul(v2(hn), v2(act["o"]), v2(tc_t))
                u["h_t"] = hn


def _prep_host(inputs):
    import ml_dtypes
    bf = ml_dtypes.bfloat16

    x = np.asarray(inputs["x"], dtype=np.float32)
    adj = np.asarray(inputs["adj"], dtype=np.float32)
    adjT = np.ascontiguousarray(adj.T).astype(bf)
    rs = adj.sum(axis=1, dtype=np.float32).reshape(1, T).astype(bf)
    wg0 = np.asarray(inputs["Wg0"], dtype=np.float32).astype(bf)
    wg1 = np.asarray(inputs["Wg1"], dtype=np.float32).astype(bf)
    bg0 = np.asarray(inputs["bg0"], dtype=np.float32).reshape(1, H).astype(bf)
    bg1 = np.asarray(inputs["bg1"], dtype=np.float32).reshape(1, H).astype(bf)
    wihT = np.ascontiguousarray(
        np.asarray(inputs["Wih"], dtype=np.float32).transpose(0, 2, 1)).astype(bf)
    whhT = np.ascontiguousarray(
        np.asarray(inputs["Whh"], dtype=np.float32).transpose(0, 2, 1)).astype(bf)
    bias = np.asarray(inputs["bih"], dtype=np.float32) + np.asarray(inputs["bhh"], dtype=np.float32)
    biasT = np.ascontiguousarray(
        bias.reshape(MAX_SKIP, 8, 128).transpose(2, 0, 1).reshape(128, MAX_SKIP * 8))
    shared = dict(adjT=adjT, rs=rs, wg0=wg0, wg1=wg1, bg0=bg0, bg1=bg1,
                  wihT=wihT, whhT=whhT, biasT=biasT)
    xb = x.astype(bf)
    in_maps = []
    for cc in range(NCORES):
        m = dict(shared)
        m["x"] = np.ascontiguousarray(xb[cc * BL:(cc + 1) * BL])
        in_maps.append(m)
    return in_maps


def get_compiled():
    global _COMPILED
    if _COMPILED is None:
        _COMPILED = _build_program()
    return _COMPILED


def kernel(**inputs) -> np.ndarray:
    from concourse.bass_utils import run_bass_kernel_spmd

    nc = get_compiled()
    in_maps = _prep_host(inputs)
    res = run_bass_kernel_spmd(nc, in_maps, list(range(NCORES)))
    outs = []
    for cc in range(NCORES):
        o = np.asarray(res.results[cc]["out"]).astype(np.float32)  # [BL, HK, 128, T]
        outs.append(o.transpose(0, 3, 1, 2).reshape(BL, T, D))
    return np.concatenate(outs, axis=0)


# revision 14
# speedup vs baseline: 1.0143x; 1.0143x over previous
"""Trainium2 Bass kernel for the DTGL GCN+windowed-LSTM module.

Computation (see reference):
  h = relu(adj @ (x @ Wg0 + bg0));  h = relu(adj @ (h @ Wg1 + bg1))
  for p in 1..4: run LSTM_p over disjoint length-p windows of h (zero init
  state), writing the last hidden state back at each window end (in place).

Sharding: pure data-parallel over batch B=64 across 8 cores (8 batches per
core); adj and all weights replicated. No collectives.

Per-core design (all matmul inputs bf16, fp32 PSUM accumulation; LSTM cell
state kept fp32 for accuracy):
  - GCN bias reassociated: adj @ (x@W + 1 b^T) = (adj@x)@W + rowsum(adj)(x)b,
    so every matmul keeps its contraction dim on partitions:
      1A: z1T[d,u] = sum_t x[t,d]*adjT[t,u]
      1B: h1[u,h]  = relu(sum_d z1T[d,u]*Wg0[d,h] + rs[u]*bg0[h])
      2A: z2T[h,u] = sum_t h1[t,h]*adjT[t,u]
      2B: h2T[h,u] = relu(sum_k Wg1[k,h]*z2T[k,u] + bg1[h]*rs[u])
  - h2T stays feature-major bf16 in SBUF; the 4 LSTM passes update it in
    place (gates via PSUM-accumulated bf16 matmuls, sigmoid/tanh on ACT with
    fused per-partition bias, cell math on DVE, window gathers on gpsimd).
  - Output is DMA'd out feature-major bf16; the host transposes/casts.

Batches run in 4 groups of 2; group g+1's GCN matmul blocks are interleaved
between group g's LSTM unit-pairs so the tensor engine stays busy while the
(scalar-engine-bound) LSTM activations drain.  LSTM chunks are processed in
pairs (A/B) with one-gate-at-a-time PSUM rotation (2 tags x 2 banks) so the
recurrent dependency never stalls the PE.
"""

import numpy as np

B, T, D, H = 64, 2048, 256, 256
MAX_SKIP = 4
NCORES = 8
BL = B // NCORES          # batches per core
GJ = 2                    # batches per group
NGRP = BL // GJ           # 4 groups
TK = T // 128             # 16 t-chunks
UC = T // 512             # 4 u-chunks of 512
HK = H // 128             # 2 feature blocks
JB0 = {"i": 0, "f": 2, "g": 4, "o": 6}
# window chunking per pass: list of (ws, cw)
CHUNKS = {
    1: [(0, 512), (512, 512), (1024, 512), (1536, 512)],
    2: [(0, 512), (512, 512)],
    3: [(0, 341), (341, 341)],
    4: [(0, 512)],
}

_COMPILED = None


def _build_program():
    import concourse.mybir as mybir
    import concourse.tile as tile
    from concourse import bacc

    f32 = mybir.dt.float32
    bf16 = mybir.dt.bfloat16

    nc = bacc.Bacc("TRN2", target_bir_lowering=False, debug=False)

    io = dict(
        x=nc.dram_tensor("x", [BL, T, D], bf16, kind="ExternalInput").ap(),
        adjT=nc.dram_tensor("adjT", [T, T], bf16, kind="ExternalInput").ap(),
        rs=nc.dram_tensor("rs", [1, T], bf16, kind="ExternalInput").ap(),
        wg0=nc.dram_tensor("wg0", [D, H], bf16, kind="ExternalInput").ap(),
        wg1=nc.dram_tensor("wg1", [D, H], bf16, kind="ExternalInput").ap(),
        bg0=nc.dram_tensor("bg0", [1, H], bf16, kind="ExternalInput").ap(),
        bg1=nc.dram_tensor("bg1", [1, H], bf16, kind="ExternalInput").ap(),
        wihT=nc.dram_tensor("wihT", [MAX_SKIP, H, 4 * H], bf16, kind="ExternalInput").ap(),
        whhT=nc.dram_tensor("whhT", [MAX_SKIP, H, 4 * H], bf16, kind="ExternalInput").ap(),
        biasT=nc.dram_tensor("biasT", [128, MAX_SKIP * 8], f32, kind="ExternalInput").ap(),
        out=nc.dram_tensor("out", [BL, HK, 128, T], bf16, kind="ExternalOutput").ap(),
    )

    with tile.TileContext(nc) as tc:
        _emit(nc, tc, mybir, io)

    nc.compile()
    return nc


class _Ctx:
    pass


def _emit(nc, tc, mybir, io):
    from contextlib import ExitStack

    f32 = mybir.dt.float32
    bf16 = mybir.dt.bfloat16
    AF = mybir.ActivationFunctionType

    c = _Ctx()
    c.nc, c.io, c.f32, c.bf16, c.AF = nc, io, f32, bf16, AF

    with ExitStack() as root:
        # ---- constants / weights ----
        cp = root.enter_context(tc.tile_pool(name="const", bufs=1))
        c.wg0_sb = cp.tile([128, HK * H], bf16, name="wg0_sb")
        c.wg1_sb = cp.tile([128, HK * H], bf16, name="wg1_sb")
        for hk in range(HK):
            nc.scalar.dma_start(out=c.wg0_sb[:, hk * H:(hk + 1) * H],
                                in_=io["wg0"][hk * 128:(hk + 1) * 128, :])
            nc.scalar.dma_start(out=c.wg1_sb[:, hk * H:(hk + 1) * H],
                                in_=io["wg1"][hk * 128:(hk + 1) * 128, :])
        c.bg0_sb = cp.tile([1, H], bf16, name="bg0_sb")
        c.bg1_sb = cp.tile([1, H], bf16, name="bg1_sb")
        c.rs_sb = cp.tile([1, T], bf16, name="rs_sb")
        c.biasT_sb = cp.tile([128, MAX_SKIP * 8], f32, name="biasT_sb")
        nc.scalar.dma_start(out=c.bg0_sb[:], in_=io["bg0"][:])
        nc.scalar.dma_start(out=c.bg1_sb[:], in_=io["bg1"][:])
        nc.scalar.dma_start(out=c.rs_sb[:], in_=io["rs"][:])
        nc.scalar.dma_start(out=c.biasT_sb[:], in_=io["biasT"][:])

        # ---- persistent pools ----
        c.h2t_pool = root.enter_context(tc.tile_pool(name="h2tp", bufs=1))
        c.h1_pool = root.enter_context(tc.tile_pool(name="h1p", bufs=1))
        c.x_pool = root.enter_context(tc.tile_pool(name="xp", bufs=1))
        c.adjt_pool = root.enter_context(tc.tile_pool(name="adjt", bufs=2))
        c.z_pool = root.enter_context(tc.tile_pool(name="zp", bufs=2))
        c.zps = root.enter_context(tc.tile_pool(name="zps", bufs=1, space="PSUM"))
        c.mps = root.enter_context(tc.tile_pool(name="mps", bufs=2, space="PSUM"))
        c.gps = root.enter_context(tc.tile_pool(name="gps", bufs=1, space="PSUM"))
        c.act_pool = root.enter_context(tc.tile_pool(name="actp", bufs=1))
        c.st_pool = root.enter_context(tc.tile_pool(name="stp", bufs=1))
        c.xc_pool = root.enter_context(tc.tile_pool(name="xcp", bufs=1))

        lwp = root.enter_context(tc.tile_pool(name="lw", bufs=1))

        # h2t slabs: 4 tags (2 groups in flight); h1: 2 tags; x: 2x16 tags
        c.h2t = {}   # batch index -> tile (assigned per group)
        c.qidx = 0   # rotating LSTM gate-psum tag

        # group 0 GCN emitted densely
        for blk in _gcn_blocks(c, tc, 0):
            blk()

        # LSTM weights (emitted after group-0 GCN so their DMA doesn't
        # compete with the critical startup x/adjT loads; scalar queue)
        c.wih = {}
        c.whh = {}
        for p in range(1, MAX_SKIP + 1):
            w = lwp.tile([128, HK * 4 * H], bf16, name=f"wih{p}")
            for hk in range(HK):
                nc.scalar.dma_start(out=w[:, hk * 4 * H:(hk + 1) * 4 * H],
                                    in_=io["wihT"][p - 1, hk * 128:(hk + 1) * 128, :])
            c.wih[p] = w
            if p > 1:
                w2 = lwp.tile([128, HK * 4 * H], bf16, name=f"whh{p}")
                for hk in range(HK):
                    nc.scalar.dma_start(out=w2[:, hk * 4 * H:(hk + 1) * 4 * H],
                                        in_=io["whhT"][p - 1, hk * 128:(hk + 1) * 128, :])
                c.whh[p] = w2

        # interleave: LSTM(g) with GCN(g+1); extra filler at pass boundaries
        for g in range(NGRP):
            gcn_next = _gcn_blocks(c, tc, g + 1) if g + 1 < NGRP else []
            gi = 0

            def pump(n):
                nonlocal gi
                for _ in range(n):
                    if gi < len(gcn_next):
                        gcn_next[gi]()
                        gi += 1
            for is_pass_end, pair in _lstm_pairs(c, tc, g):
                pair()
                pump(4 if is_pass_end else 1)
            pump(len(gcn_next))
            # output DMA for this group's batches
            for jj in range(GJ):
                b = g * GJ + jj
                slab = c.h2t[b]
                for hk in range(HK):
                    nc.gpsimd.dma_start(out=io["out"][b, hk],
                                        in_=slab[:, hk * T:(hk + 1) * T])


def _gcn_blocks(c, tc, g):
    """Return list of emission callbacks for group g's GCN (both layers)."""
    if g >= NGRP:
        return []
    nc, io, f32, bf16, AF = c.nc, c.io, c.f32, c.bf16, c.AF
    blocks = []

    xs = {}
    h1 = {}
    adjt = {}
    loaded = set()

    def load_adjt(u4, phase):
        if (u4, phase) in loaded:
            return
        loaded.add((u4, phase))
        us = u4 * 512
        for k in range(TK):
            a = c.adjt_pool.tile([128, 512], bf16, name=f"adjt_{k}", tag=f"a_{k}")
            nc.sync.dma_start(out=a[:], in_=io["adjT"][k * 128:(k + 1) * 128, us:us + 512])
            adjt[k] = a

    def load_x():
        for jj in range(GJ):
            b = g * GJ + jj
            h1[jj] = c.h1_pool.tile([128, TK * H], bf16, name=f"h1_{jj}",
                                    tag=f"h1_{jj}")
            for k in range(TK):
                xt = c.x_pool.tile([128, D], bf16, name="xt", tag=f"x_{jj}_{k}")
                nc.sync.dma_start(out=xt[:], in_=io["x"][b, k * 128:(k + 1) * 128, :])
                xs[(jj, k)] = xt
            c.h2t[b] = c.h2t_pool.tile([128, HK * T], bf16, name=f"h2t_{b % 4}",
                                       tag=f"h2t_{b % 4}")
    blocks.append(lambda: load_adjt(0, 1))
    blocks.append(load_x)

    def ph1_block(u4, jj):
        if jj == 0:
            load_adjt(u4, 1)
        us = u4 * 512
        zp = c.zps.tile([128, 1024], f32, name="zp", tag="zp")
        for k in range(TK):
            for dk in range(HK):
                nc.tensor.matmul(zp[:, dk * 512:(dk + 1) * 512],
                                 xs[(jj, k)][:, dk * 128:(dk + 1) * 128],
                                 adjt[k][:],
                                 start=(k == 0), stop=(k == TK - 1))
        z1t = c.z_pool.tile([128, 1024], bf16, name="z1t", tag="z1t")
        nc.vector.tensor_copy(z1t[:], zp[:])
        for ub in range(4):
            ua = us + ub * 128
            hp = c.mps.tile([128, 512], f32, name="hp", tag="mp")
            for dk in range(HK):
                nc.tensor.matmul(hp[:, :H],
                                 z1t[:, dk * 512 + ub * 128: dk * 512 + (ub + 1) * 128],
                                 c.wg0_sb[:, dk * H:(dk + 1) * H],
                                 start=(dk == 0), stop=False)
            nc.tensor.matmul(hp[:, :H], c.rs_sb[0:1, ua:ua + 128], c.bg0_sb[0:1, :],
                             start=False, stop=True)
            nc.vector.tensor_relu(h1[jj][:, (u4 * 4 + ub) * H:(u4 * 4 + ub + 1) * H],
                                  hp[:, :H])

    def ph2_block(u4, jj):
        if jj == 0:
            load_adjt(u4, 2)
        us = u4 * 512
        b = g * GJ + jj
        slab = c.h2t[b]
        zp = c.zps.tile([128, 1024], f32, name="zp2", tag="zp")
        for hk in range(HK):
            for k in range(TK):
                nc.tensor.matmul(zp[:, hk * 512:(hk + 1) * 512],
                                 h1[jj][:, k * H + hk * 128: k * H + (hk + 1) * 128],
                                 adjt[k][:],
                                 start=(k == 0), stop=(k == TK - 1))
        z2t = c.z_pool.tile([128, 1024], bf16, name="z2t", tag="z2t")
        nc.vector.tensor_copy(z2t[:], zp[:])
        for ho in range(HK):
            hp = c.mps.tile([128, 512], f32, name="hp2", tag="mp")
            for hk in range(HK):
                nc.tensor.matmul(hp[:],
                                 c.wg1_sb[:, hk * H + ho * 128: hk * H + (ho + 1) * 128],
                                 z2t[:, hk * 512:(hk + 1) * 512],
                                 start=(hk == 0), stop=False)
            nc.tensor.matmul(hp[:], c.bg1_sb[0:1, ho * 128:(ho + 1) * 128],
                             c.rs_sb[0:1, us:us + 512], start=False, stop=True)
            nc.vector.tensor_relu(slab[:, ho * T + us: ho * T + us + 512], hp[:])

    for u4 in range(UC):
        for jj in range(GJ):
            blocks.append(lambda u4=u4, jj=jj: ph1_block(u4, jj))
    for u4 in range(UC):
        for jj in range(GJ):
            blocks.append(lambda u4=u4, jj=jj: ph2_block(u4, jj))
    return blocks


def _lstm_pairs(c, tc, g):
    """Yield (is_pass_end, callback), one per unit-pair, passes p=1..4."""
    for p in range(1, MAX_SKIP + 1):
        units = [(jj, ws, cw) for jj in range(GJ) for (ws, cw) in CHUNKS[p]]
        for i in range(0, len(units), 2):
            pair = units[i:i + 2]
            yield (i + 2 >= len(units),
                   lambda p=p, pair=tuple(pair): _emit_pair(c, tc, g, p, pair))


def _emit_pair(c, tc, g, p, pair):
    nc, f32, bf16, AF = c.nc, c.f32, c.bf16, c.AF
    nw = T // p

    # per-unit state dicts
    st = []
    for slot, (jj, ws, cw) in enumerate(pair):
        b = g * GJ + jj
        slab = c.h2t[b]
        view = [slab[:, hk * T: hk * T + nw * p].rearrange("a (w q) -> a w q", q=p)
                for hk in range(HK)]
        u = dict(slot=slot, jj=jj, ws=ws, cw=cw, view=view, slab=slab,
                 c_t=None, h_t=None, xc={})
        st.append(u)

    # prefetch gathers for all steps (gpsimd); pass>1 only
    if p > 1:
        for u in st:
            for t in range(p):
                xc = c.xc_pool.tile([128, 1024], bf16, name="xc",
                                    tag=f"xc{u['slot']}_{t}")
                for hk in range(HK):
                    nc.gpsimd.tensor_copy(
                        xc[:, hk * 512: hk * 512 + u["cw"]],
                        u["view"][hk][:, u["ws"]:u["ws"] + u["cw"], t:t + 1])
                u["xc"][t] = xc

    for t in range(p):
        gates = "igo" if t == 0 else "ifgo"
        for u in st:
            cw, ws, slot = u["cw"], u["ws"], u["slot"]
            n2 = [slice(0, cw), slice(512, 512 + cw)]
            act = {}
            for gn in gates:
                # one gate = one psum quarter (rotating tag)
                q = c.gps.tile([128, 1024], f32, name="q", tag=f"q{c.qidx % 2}")
                c.qidx += 1
                for half in range(2):
                    jb = JB0[gn] + half
                    o = q[:, half * 512: half * 512 + cw]
                    for hk in range(HK):
                        if p == 1:
                            rhs = u["slab"][:, hk * T + ws: hk * T + ws + cw]
                        else:
                            rhs = u["xc"][t][:, hk * 512: hk * 512 + cw]
                        nc.tensor.matmul(
                            o,
                            c.wih[p][:, hk * 4 * H + jb * 128: hk * 4 * H + (jb + 1) * 128],
                            rhs,
                            start=(hk == 0),
                            stop=(t == 0 and hk == HK - 1))
                    if t > 0:
                        for hk in range(HK):
                            nc.tensor.matmul(
                                o,
                                c.whh[p][:, hk * 4 * H + jb * 128: hk * 4 * H + (jb + 1) * 128],
                                u["h_t"][:, hk * 512: hk * 512 + cw],
                                start=False, stop=(hk == HK - 1))
                # activations straight after this gate's matmuls
                fn = AF.Tanh if gn == "g" else AF.Sigmoid
                a = c.act_pool.tile([128, 1024], bf16, name=f"a_{gn}",
                                    tag=f"a{slot}_{gn}")
                act[gn] = a
                for half in range(2):
                    col = (p - 1) * 8 + JB0[gn] + half
                    nc.scalar.activation(
                        a[:, half * 512: half * 512 + cw],
                        q[:, half * 512: half * 512 + cw],
                        fn, bias=c.biasT_sb[:, col:col + 1])

            # cell math (DVE); strided [128, 2, cw] views over the 1024 tiles
            def v2(tile):
                return tile.rearrange("a (h w) -> a h w", h=2)[:, :, :cw]
            if t == 0:
                cn = c.st_pool.tile([128, 1024], f32, name="cn", tag=f"c{slot}")
                nc.vector.tensor_mul(v2(cn), v2(act["i"]), v2(act["g"]))
                u["c_t"] = cn
            else:
                ig = c.st_pool.tile([128, 1024], bf16, name="ig", tag=f"ig{slot}")
                nc.vector.tensor_mul(v2(ig), v2(act["i"]), v2(act["g"]))
                cn = u["c_t"]
                nc.vector.tensor_mul(v2(cn), v2(act["f"]), v2(cn))
                nc.vector.tensor_add(v2(cn), v2(cn), v2(ig))
            # tanh(c) on ACT (single strided instr, no bias)
            tc_t = c.st_pool.tile([128, 1024], bf16, name="tc", tag=f"tc{slot}",
                                  bufs=2)
            nc.scalar.activation(v2(tc_t), v2(u["c_t"]), AF.Tanh)
            if t == p - 1:
                for hk in range(HK):
                    nc.vector.tensor_mul(
                        u["view"][hk][:, ws:ws + cw, p - 1:p],
                        act["o"][:, hk * 512: hk * 512 + cw],
                        tc_t[:, hk * 512: hk * 512 + cw])
            else:
                hn = c.st_pool.tile([128, 1024], bf16, name="hn", tag=f"h{slot}",
                                    bufs=2)
                nc.vector.tensor_mul(v2(hn), v2(act["o"]), v2(tc_t))
                u["h_t"] = hn


def _prep_host(inputs):
    import ml_dtypes
    bf = ml_dtypes.bfloat16

    x = np.asarray(inputs["x"], dtype=np.float32)
    adj = np.asarray(inputs["adj"], dtype=np.float32)
    adjT = np.ascontiguousarray(adj.T).astype(bf)
    rs = adj.sum(axis=1, dtype=np.float32).reshape(1, T).astype(bf)
    wg0 = np.asarray(inputs["Wg0"], dtype=np.float32).astype(bf)
    wg1 = np.asarray(inputs["Wg1"], dtype=np.float32).astype(bf)
    bg0 = np.asarray(inputs["bg0"], dtype=np.float32).reshape(1, H).astype(bf)
    bg1 = np.asarray(inputs["bg1"], dtype=np.float32).reshape(1, H).astype(bf)
    wihT = np.ascontiguousarray(
        np.asarray(inputs["Wih"], dtype=np.float32).transpose(0, 2, 1)).astype(bf)
    whhT = np.ascontiguousarray(
        np.asarray(inputs["Whh"], dtype=np.float32).transpose(0, 2, 1)).astype(bf)
    bias = np.asarray(inputs["bih"], dtype=np.float32) + np.asarray(inputs["bhh"], dtype=np.float32)
    biasT = np.ascontiguousarray(
        bias.reshape(MAX_SKIP, 8, 128).transpose(2, 0, 1).reshape(128, MAX_SKIP * 8))
    shared = dict(adjT=adjT, rs=rs, wg0=wg0, wg1=wg1, bg0=bg0, bg1=bg1,
                  wihT=wihT, whhT=whhT, biasT=biasT)
    xb = x.astype(bf)
    in_maps = []
    for cc in range(NCORES):
        m = dict(shared)
        m["x"] = np.ascontiguousarray(xb[cc * BL:(cc + 1) * BL])
        in_maps.append(m)
    return in_maps


def get_compiled():
    global _COMPILED
    if _COMPILED is None:
        _COMPILED = _build_program()
    return _COMPILED


def kernel(**inputs) -> np.ndarray:
    from concourse.bass_utils import run_bass_kernel_spmd

    nc = get_compiled()
    in_maps = _prep_host(inputs)
    res = run_bass_kernel_spmd(nc, in_maps, list(range(NCORES)))
    outs = []
    for cc in range(NCORES):
        o = np.asarray(res.results[cc]["out"]).astype(np.float32)  # [BL, HK, 128, T]
        outs.append(o.transpose(0, 3, 1, 2).reshape(BL, T, D))
    return np.concatenate(outs, axis=0)


# revision 19
# speedup vs baseline: 1.2026x; 1.1857x over previous
"""Trainium2 Bass kernel for the DTGL GCN+windowed-LSTM module.

Computation (see reference):
  h = relu(adj @ (x @ Wg0 + bg0));  h = relu(adj @ (h @ Wg1 + bg1))
  for p in 1..4: run LSTM_p over disjoint length-p windows of h (zero init
  state), writing the last hidden state back at each window end (in place).

Sharding: pure data-parallel over batch B=64 across 8 cores (8 batches per
core); adj and all weights replicated. No collectives.

Per-core design (all matmul inputs bf16, fp32 PSUM accumulation; LSTM cell
state kept fp32 for accuracy):
  - GCN bias reassociated: adj @ (x@W + 1 b^T) = (adj@x)@W + rowsum(adj)(x)b,
    so every matmul keeps its contraction dim on partitions:
      1A: z1T[d,u] = sum_t x[t,d]*adjT[t,u]
      1B: h1[u,h]  = relu(sum_d z1T[d,u]*Wg0[d,h] + rs[u]*bg0[h])
      2A: z2T[h,u] = sum_t h1[t,h]*adjT[t,u]
      2B: h2T[h,u] = relu(sum_k Wg1[k,h]*z2T[k,u] + bg1[h]*rs[u])
  - h2T stays feature-major bf16 in SBUF; the 4 LSTM passes update it in
    place (gates via PSUM-accumulated bf16 matmuls, sigmoid/tanh on ACT with
    fused per-partition bias, cell math on DVE, window gathers on gpsimd).
  - Output is DMA'd out feature-major bf16; the host transposes/casts.

Batches run in 4 groups of 2; group g+1's GCN matmul blocks are interleaved
between group g's LSTM unit-pairs so the tensor engine stays busy while the
(scalar-engine-bound) LSTM activations drain.  LSTM chunks are processed in
pairs (A/B) with one-gate-at-a-time PSUM rotation (2 tags x 2 banks) so the
recurrent dependency never stalls the PE.
"""

import numpy as np

B, T, D, H = 64, 2048, 256, 256
MAX_SKIP = 4
NCORES = 8
BL = B // NCORES          # batches per core
GJ = 2                    # batches per group
NGRP = BL // GJ           # 4 groups
TK = T // 128             # 16 t-chunks
UC = T // 512             # 4 u-chunks of 512
HK = H // 128             # 2 feature blocks
JB0 = {"i": 0, "f": 2, "g": 4, "o": 6}
# window chunking per pass: list of (ws, cw)
CHUNKS = {
    1: [(0, 512), (512, 512), (1024, 512), (1536, 512)],
    2: [(0, 512), (512, 512)],
    3: [(0, 341), (341, 341)],
    4: [(0, 512)],
}

_COMPILED = None


def _build_program():
    import concourse.mybir as mybir
    import concourse.tile as tile
    from concourse import bacc

    f32 = mybir.dt.float32
    bf16 = mybir.dt.bfloat16

    nc = bacc.Bacc("TRN2", target_bir_lowering=False, debug=False)

    f8 = mybir.dt.float8e4
    io = dict(
        x=nc.dram_tensor("x", [BL, T, D], f8, kind="ExternalInput").ap(),
        adjT=nc.dram_tensor("adjT", [T, T], f8, kind="ExternalInput").ap(),
        rs=nc.dram_tensor("rs", [1, T], bf16, kind="ExternalInput").ap(),
        wg0=nc.dram_tensor("wg0", [D, H], bf16, kind="ExternalInput").ap(),
        wg1=nc.dram_tensor("wg1", [D, H], bf16, kind="ExternalInput").ap(),
        bg0=nc.dram_tensor("bg0", [1, H], bf16, kind="ExternalInput").ap(),
        bg1=nc.dram_tensor("bg1", [1, H], bf16, kind="ExternalInput").ap(),
        wihT=nc.dram_tensor("wihT", [MAX_SKIP, H, 4 * H], bf16, kind="ExternalInput").ap(),
        whhT=nc.dram_tensor("whhT", [MAX_SKIP, H, 4 * H], bf16, kind="ExternalInput").ap(),
        biasT=nc.dram_tensor("biasT", [128, MAX_SKIP * 8], f32, kind="ExternalInput").ap(),
        out=nc.dram_tensor("out", [BL, HK, 128, T], bf16, kind="ExternalOutput").ap(),
    )

    with tile.TileContext(nc) as tc:
        _emit(nc, tc, mybir, io)

    nc.compile()
    return nc


class _Ctx:
    pass


def _emit(nc, tc, mybir, io):
    from contextlib import ExitStack

    f32 = mybir.dt.float32
    bf16 = mybir.dt.bfloat16
    AF = mybir.ActivationFunctionType

    c = _Ctx()
    c.nc, c.io, c.f32, c.bf16, c.AF = nc, io, f32, bf16, AF
    c.f8 = mybir.dt.float8e4
    c.DR = mybir.MatmulPerfMode.DoubleRow
    c.ALU = mybir.AluOpType

    with ExitStack() as root:
        # ---- constants / weights ----
        cp = root.enter_context(tc.tile_pool(name="const", bufs=1))
        c.wg0_sb = cp.tile([128, HK * H], bf16, name="wg0_sb")
        c.wg1_sb = cp.tile([128, HK * H], bf16, name="wg1_sb")
        for hk in range(HK):
            nc.scalar.dma_start(out=c.wg0_sb[:, hk * H:(hk + 1) * H],
                                in_=io["wg0"][hk * 128:(hk + 1) * 128, :])
            nc.scalar.dma_start(out=c.wg1_sb[:, hk * H:(hk + 1) * H],
                                in_=io["wg1"][hk * 128:(hk + 1) * 128, :])
        c.bg0_sb = cp.tile([1, H], bf16, name="bg0_sb")
        c.bg1_sb = cp.tile([1, H], bf16, name="bg1_sb")
        c.rs_sb = cp.tile([1, T], bf16, name="rs_sb")
        c.biasT_sb = cp.tile([128, MAX_SKIP * 8], f32, name="biasT_sb")
        nc.scalar.dma_start(out=c.bg0_sb[:], in_=io["bg0"][:])
        nc.scalar.dma_start(out=c.bg1_sb[:], in_=io["bg1"][:])
        nc.scalar.dma_start(out=c.rs_sb[:], in_=io["rs"][:])
        nc.scalar.dma_start(out=c.biasT_sb[:], in_=io["biasT"][:])

        # ---- persistent pools ----
        c.h2t_pool = root.enter_context(tc.tile_pool(name="h2tp", bufs=1))
        c.h1_pool = root.enter_context(tc.tile_pool(name="h1p", bufs=1))
        c.x_pool = root.enter_context(tc.tile_pool(name="xp", bufs=1))
        c.adjt_pool = root.enter_context(tc.tile_pool(name="adjt", bufs=2))
        c.z_pool = root.enter_context(tc.tile_pool(name="zp", bufs=2))
        c.zps = root.enter_context(tc.tile_pool(name="zps", bufs=1, space="PSUM"))
        c.mps = root.enter_context(tc.tile_pool(name="mps", bufs=2, space="PSUM"))
        c.gps = root.enter_context(tc.tile_pool(name="gps", bufs=1, space="PSUM"))
        c.act_pool = root.enter_context(tc.tile_pool(name="actp", bufs=1))
        c.st_pool = root.enter_context(tc.tile_pool(name="stp", bufs=1))
        c.xc_pool = root.enter_context(tc.tile_pool(name="xcp", bufs=1))

        lwp = root.enter_context(tc.tile_pool(name="lw", bufs=1))

        # h2t slabs: 4 tags (2 groups in flight); h1: 2 tags; x: 2x16 tags
        c.h2t = {}   # batch index -> tile (assigned per group)
        c.qidx = 0   # rotating LSTM gate-psum tag

        # group 0 GCN emitted densely
        for blk in _gcn_blocks(c, tc, 0):
            blk()

        # LSTM weights (emitted after group-0 GCN so their DMA doesn't
        # compete with the critical startup x/adjT loads; scalar queue)
        c.wih = {}
        c.whh = {}
        for p in range(1, MAX_SKIP + 1):
            w = lwp.tile([128, HK * 4 * H], bf16, name=f"wih{p}")
            for hk in range(HK):
                nc.scalar.dma_start(out=w[:, hk * 4 * H:(hk + 1) * 4 * H],
                                    in_=io["wihT"][p - 1, hk * 128:(hk + 1) * 128, :])
            c.wih[p] = w
            if p > 1:
                w2 = lwp.tile([128, HK * 4 * H], bf16, name=f"whh{p}")
                for hk in range(HK):
                    nc.scalar.dma_start(out=w2[:, hk * 4 * H:(hk + 1) * 4 * H],
                                        in_=io["whhT"][p - 1, hk * 128:(hk + 1) * 128, :])
                c.whh[p] = w2

        # interleave: LSTM(g) with GCN(g+1); extra filler at pass boundaries
        for g in range(NGRP):
            gcn_next = _gcn_blocks(c, tc, g + 1) if g + 1 < NGRP else []
            gi = 0

            def pump(n):
                nonlocal gi
                for _ in range(n):
                    if gi < len(gcn_next):
                        gcn_next[gi]()
                        gi += 1
            for is_pass_end, pair in _lstm_pairs(c, tc, g):
                pair()
                pump(4 if is_pass_end else 1)
            pump(len(gcn_next))
            # output DMA for this group's batches
            for jj in range(GJ):
                b = g * GJ + jj
                slab = c.h2t[b]
                for hk in range(HK):
                    nc.gpsimd.dma_start(out=io["out"][b, hk],
                                        in_=slab[:, hk * T:(hk + 1) * T])


def _gcn_blocks(c, tc, g):
    """Return list of emission callbacks for group g's GCN (both layers)."""
    if g >= NGRP:
        return []
    nc, io, f32, bf16, AF = c.nc, c.io, c.f32, c.bf16, c.AF
    blocks = []

    KC = TK // 2   # 8 double-row contraction chunks of 256
    xs = {}
    h1 = {}
    adjt = {}
    loaded = set()
    f8 = c.f8

    def load_adjt_k(u4, kc):
        us = u4 * 512
        a = c.adjt_pool.tile([128, 1024], f8, name=f"adjt_{kc}", tag=f"a_{kc}")
        nc.sync.dma_start(
            out=a.rearrange("p (a u) -> p a u", a=2),
            in_=io["adjT"][kc * 256:(kc + 1) * 256, us:us + 512]
            .rearrange("(a p) u -> p a u", a=2))
        adjt[kc] = a

    def load_adjt(u4, phase):
        if (u4, phase) in loaded:
            return
        loaded.add((u4, phase))
        for kc in range(KC):
            load_adjt_k(u4, kc)

    def load_x_k(jj, kc):
        b = g * GJ + jj
        xt = c.x_pool.tile([128, 512], f8, name="xt", tag=f"x_{jj}_{kc}")
        nc.sync.dma_start(
            out=xt.rearrange("p (a d) -> p a d", a=2),
            in_=io["x"][b, kc * 256:(kc + 1) * 256, :]
            .rearrange("(a p) d -> p a d", a=2))
        xs[(jj, kc)] = xt

    def load_first():
        # startup-critical: interleave adjT(u0) / x(jj0) in consumption order
        loaded.add((0, 1))
        for jj in range(GJ):
            b = g * GJ + jj
            h1[jj] = c.h1_pool.tile([128, TK * H], f8, name=f"h1_{jj}",
                                    tag=f"h1_{jj}")
            c.h2t[b] = c.h2t_pool.tile([128, HK * T], bf16, name=f"h2t_{b % 4}",
                                       tag=f"h2t_{b % 4}")
        for kc in range(KC):
            load_adjt_k(0, kc)
            load_x_k(0, kc)
        for kc in range(KC):
            load_x_k(1, kc)
    blocks.append(load_first)

    def ph1_block(u4, jj):
        if jj == 0:
            load_adjt(u4, 1)
        us = u4 * 512
        zp = c.zps.tile([128, 1024], f32, name="zp", tag="zp")
        for kc in range(KC):
            xv = xs[(jj, kc)].rearrange("p (a d) -> p a d", a=2)
            av = adjt[kc].rearrange("p (a u) -> p a u", a=2)
            for dk in range(HK):
                nc.tensor.matmul(zp[:, dk * 512:(dk + 1) * 512],
                                 xv[:, :, dk * 128:(dk + 1) * 128],
                                 av,
                                 start=(kc == 0), stop=(kc == KC - 1),
                                 perf_mode=c.DR)
        z1t = c.z_pool.tile([128, 1024], bf16, name="z1t", tag="z1t")
        nc.vector.tensor_copy(z1t[:], zp[:])
        for ub in range(4):
            ua = us + ub * 128
            hp = c.mps.tile([128, 512], f32, name="hp", tag="mp")
            for dk in range(HK):
                nc.tensor.matmul(hp[:, :H],
                                 z1t[:, dk * 512 + ub * 128: dk * 512 + (ub + 1) * 128],
                                 c.wg0_sb[:, dk * H:(dk + 1) * H],
                                 start=(dk == 0), stop=False)
            nc.tensor.matmul(hp[:, :H], c.rs_sb[0:1, ua:ua + 128], c.bg0_sb[0:1, :],
                             start=False, stop=True)
            # h1 stored as fp8(relu(.)*8); the 1/8 is folded into Wg1
            nc.vector.tensor_scalar(
                h1[jj][:, (u4 * 4 + ub) * H:(u4 * 4 + ub + 1) * H],
                hp[:, :H], 8.0, 0.0, op0=c.ALU.mult, op1=c.ALU.max)

    def ph2_block(u4, jj):
        if jj == 0:
            load_adjt(u4, 2)
        us = u4 * 512
        b = g * GJ + jj
        slab = c.h2t[b]
        zp = c.zps.tile([128, 1024], f32, name="zp2", tag="zp")
        for hk in range(HK):
            for kc in range(KC):
                hv = h1[jj][:, kc * 512:(kc + 1) * 512].rearrange(
                    "p (a h) -> p a h", a=2)
                av = adjt[kc].rearrange("p (a u) -> p a u", a=2)
                nc.tensor.matmul(zp[:, hk * 512:(hk + 1) * 512],
                                 hv[:, :, hk * 128:(hk + 1) * 128],
                                 av,
                                 start=(kc == 0), stop=(kc == KC - 1),
                                 perf_mode=c.DR)
        z2t = c.z_pool.tile([128, 1024], bf16, name="z2t", tag="z2t")
        nc.vector.tensor_copy(z2t[:], zp[:])
        for ho in range(HK):
            hp = c.mps.tile([128, 512], f32, name="hp2", tag="mp")
            for hk in range(HK):
                nc.tensor.matmul(hp[:],
                                 c.wg1_sb[:, hk * H + ho * 128: hk * H + (ho + 1) * 128],
                                 z2t[:, hk * 512:(hk + 1) * 512],
                                 start=(hk == 0), stop=False)
            nc.tensor.matmul(hp[:], c.bg1_sb[0:1, ho * 128:(ho + 1) * 128],
                             c.rs_sb[0:1, us:us + 512], start=False, stop=True)
            nc.vector.tensor_relu(slab[:, ho * T + us: ho * T + us + 512], hp[:])

    for u4 in range(UC):
        for jj in range(GJ):
            blocks.append(lambda u4=u4, jj=jj: ph1_block(u4, jj))
    for u4 in range(UC):
        for jj in range(GJ):
            blocks.append(lambda u4=u4, jj=jj: ph2_block(u4, jj))
    return blocks


def _lstm_pairs(c, tc, g):
    """Yield (is_pass_end, callback), one per unit-pair, passes p=1..4."""
    for p in range(1, MAX_SKIP + 1):
        units = [(jj, ws, cw) for jj in range(GJ) for (ws, cw) in CHUNKS[p]]
        for i in range(0, len(units), 2):
            pair = units[i:i + 2]
            yield (i + 2 >= len(units),
                   lambda p=p, pair=tuple(pair): _emit_pair(c, tc, g, p, pair))


def _emit_pair(c, tc, g, p, pair):
    nc, f32, bf16, AF = c.nc, c.f32, c.bf16, c.AF
    nw = T // p

    # per-unit state dicts
    st = []
    for slot, (jj, ws, cw) in enumerate(pair):
        b = g * GJ + jj
        slab = c.h2t[b]
        view = [slab[:, hk * T: hk * T + nw * p].rearrange("a (w q) -> a w q", q=p)
                for hk in range(HK)]
        u = dict(slot=slot, jj=jj, ws=ws, cw=cw, view=view, slab=slab,
                 c_t=None, h_t=None, xc={})
        st.append(u)

    # prefetch gathers for all steps (gpsimd); pass>1 only
    if p > 1:
        for u in st:
            for t in range(p):
                xc = c.xc_pool.tile([128, 1024], bf16, name="xc",
                                    tag=f"xc{u['slot']}_{t}")
                for hk in range(HK):
                    nc.gpsimd.tensor_copy(
                        xc[:, hk * 512: hk * 512 + u["cw"]],
                        u["view"][hk][:, u["ws"]:u["ws"] + u["cw"], t:t + 1])
                u["xc"][t] = xc

    for t in range(p):
        gates = "igo" if t == 0 else "ifgo"
        for u in st:
            cw, ws, slot = u["cw"], u["ws"], u["slot"]
            n2 = [slice(0, cw), slice(512, 512 + cw)]
            act = {}
            for gn in gates:
                # one gate = one psum quarter (rotating tag)
                q = c.gps.tile([128, 1024], f32, name="q", tag=f"q{c.qidx % 2}")
                c.qidx += 1
                for half in range(2):
                    jb = JB0[gn] + half
                    o = q[:, half * 512: half * 512 + cw]
                    for hk in range(HK):
                        if p == 1:
                            rhs = u["slab"][:, hk * T + ws: hk * T + ws + cw]
                        else:
                            rhs = u["xc"][t][:, hk * 512: hk * 512 + cw]
                        nc.tensor.matmul(
                            o,
                            c.wih[p][:, hk * 4 * H + jb * 128: hk * 4 * H + (jb + 1) * 128],
                            rhs,
                            start=(hk == 0),
                            stop=(t == 0 and hk == HK - 1))
                    if t > 0:
                        for hk in range(HK):
                            nc.tensor.matmul(
                                o,
                                c.whh[p][:, hk * 4 * H + jb * 128: hk * 4 * H + (jb + 1) * 128],
                                u["h_t"][:, hk * 512: hk * 512 + cw],
                                start=False, stop=(hk == HK - 1))
                # activations straight after this gate's matmuls
                fn = AF.Tanh if gn == "g" else AF.Sigmoid
                a = c.act_pool.tile([128, 1024], bf16, name=f"a_{gn}",
                                    tag=f"a{slot}_{gn}")
                act[gn] = a
                for half in range(2):
                    col = (p - 1) * 8 + JB0[gn] + half
                    nc.scalar.activation(
                        a[:, half * 512: half * 512 + cw],
                        q[:, half * 512: half * 512 + cw],
                        fn, bias=c.biasT_sb[:, col:col + 1])

            # cell math (DVE); strided [128, 2, cw] views over the 1024 tiles
            def v2(tile):
                return tile.rearrange("a (h w) -> a h w", h=2)[:, :, :cw]
            if t == 0:
                cn = c.st_pool.tile([128, 1024], f32, name="cn", tag=f"c{slot}")
                nc.vector.tensor_mul(v2(cn), v2(act["i"]), v2(act["g"]))
                u["c_t"] = cn
            else:
                ig = c.st_pool.tile([128, 1024], bf16, name="ig", tag=f"ig{slot}")
                nc.vector.tensor_mul(v2(ig), v2(act["i"]), v2(act["g"]))
                cn = u["c_t"]
                nc.vector.tensor_mul(v2(cn), v2(act["f"]), v2(cn))
                nc.vector.tensor_add(v2(cn), v2(cn), v2(ig))
            # tanh(c) on ACT (single strided instr, no bias)
            tc_t = c.st_pool.tile([128, 1024], bf16, name="tc", tag=f"tc{slot}",
                                  bufs=2)
            nc.scalar.activation(v2(tc_t), v2(u["c_t"]), AF.Tanh)
            if t == p - 1:
                for hk in range(HK):
                    nc.vector.tensor_mul(
                        u["view"][hk][:, ws:ws + cw, p - 1:p],
                        act["o"][:, hk * 512: hk * 512 + cw],
                        tc_t[:, hk * 512: hk * 512 + cw])
            else:
                hn = c.st_pool.tile([128, 1024], bf16, name="hn", tag=f"h{slot}",
                                    bufs=2)
                nc.vector.tensor_mul(v2(hn), v2(act["o"]), v2(tc_t))
                u["h_t"] = hn


SA = 1024.0   # adjT fp8 prescale (folded out of Wg0/Wg1)
SH = 8.0      # h1 fp8 prescale (folded out of Wg1)


def _prep_host(inputs):
    import ml_dtypes
    bf = ml_dtypes.bfloat16
    f8 = ml_dtypes.float8_e4m3fn

    x = np.asarray(inputs["x"], dtype=np.float32)
    adj = np.asarray(inputs["adj"], dtype=np.float32)
    adjT = np.ascontiguousarray(adj.T * SA).astype(f8)
    rs = adj.sum(axis=1, dtype=np.float32).reshape(1, T).astype(bf)
    wg0 = (np.asarray(inputs["Wg0"], dtype=np.float32) / SA).astype(bf)
    wg1 = (np.asarray(inputs["Wg1"], dtype=np.float32) / (SA * SH)).astype(bf)
    bg0 = np.asarray(inputs["bg0"], dtype=np.float32).reshape(1, H).astype(bf)
    bg1 = np.asarray(inputs["bg1"], dtype=np.float32).reshape(1, H).astype(bf)
    wihT = np.ascontiguousarray(
        np.asarray(inputs["Wih"], dtype=np.float32).transpose(0, 2, 1)).astype(bf)
    whhT = np.ascontiguousarray(
        np.asarray(inputs["Whh"], dtype=np.float32).transpose(0, 2, 1)).astype(bf)
    bias = np.asarray(inputs["bih"], dtype=np.float32) + np.asarray(inputs["bhh"], dtype=np.float32)
    biasT = np.ascontiguousarray(
        bias.reshape(MAX_SKIP, 8, 128).transpose(2, 0, 1).reshape(128, MAX_SKIP * 8))
    shared = dict(adjT=adjT, rs=rs, wg0=wg0, wg1=wg1, bg0=bg0, bg1=bg1,
                  wihT=wihT, whhT=whhT, biasT=biasT)
    xb = x.astype(f8)
    in_maps = []
    for cc in range(NCORES):
        m = dict(shared)
        m["x"] = np.ascontiguousarray(xb[cc * BL:(cc + 1) * BL])
        in_maps.append(m)
    return in_maps


def get_compiled():
    global _COMPILED
    if _COMPILED is None:
        _COMPILED = _build_program()
    return _COMPILED


def kernel(**inputs) -> np.ndarray:
    from concourse.bass_utils import run_bass_kernel_spmd

    nc = get_compiled()
    in_maps = _prep_host(inputs)
    res = run_bass_kernel_spmd(nc, in_maps, list(range(NCORES)))
    outs = []
    for cc in range(NCORES):
        o = np.asarray(res.results[cc]["out"]).astype(np.float32)  # [BL, HK, 128, T]
        outs.append(o.transpose(0, 3, 1, 2).reshape(BL, T, D))
    return np.concatenate(outs, axis=0)


# revision 21
# speedup vs baseline: 1.2511x; 1.0403x over previous
"""Trainium2 Bass kernel for the DTGL GCN+windowed-LSTM module.

Computation (see reference):
  h = relu(adj @ (x @ Wg0 + bg0));  h = relu(adj @ (h @ Wg1 + bg1))
  for p in 1..4: run LSTM_p over disjoint length-p windows of h (zero init
  state), writing the last hidden state back at each window end (in place).

Sharding: pure data-parallel over batch B=64 across 8 cores (8 batches per
core); adj and all weights replicated. No collectives.

Per-core design (all matmul inputs bf16, fp32 PSUM accumulation; LSTM cell
state kept fp32 for accuracy):
  - GCN bias reassociated: adj @ (x@W + 1 b^T) = (adj@x)@W + rowsum(adj)(x)b,
    so every matmul keeps its contraction dim on partitions:
      1A: z1T[d,u] = sum_t x[t,d]*adjT[t,u]
      1B: h1[u,h]  = relu(sum_d z1T[d,u]*Wg0[d,h] + rs[u]*bg0[h])
      2A: z2T[h,u] = sum_t h1[t,h]*adjT[t,u]
      2B: h2T[h,u] = relu(sum_k Wg1[k,h]*z2T[k,u] + bg1[h]*rs[u])
  - h2T stays feature-major bf16 in SBUF; the 4 LSTM passes update it in
    place (gates via PSUM-accumulated bf16 matmuls, sigmoid/tanh on ACT with
    fused per-partition bias, cell math on DVE, window gathers on gpsimd).
  - Output is DMA'd out feature-major bf16; the host transposes/casts.

Batches run in 4 groups of 2; group g+1's GCN matmul blocks are interleaved
between group g's LSTM unit-pairs so the tensor engine stays busy while the
(scalar-engine-bound) LSTM activations drain.  LSTM chunks are processed in
pairs (A/B) with one-gate-at-a-time PSUM rotation (2 tags x 2 banks) so the
recurrent dependency never stalls the PE.
"""

import numpy as np

B, T, D, H = 64, 2048, 256, 256
MAX_SKIP = 4
NCORES = 8
BL = B // NCORES          # batches per core
GJ = 2                    # batches per group
NGRP = BL // GJ           # 4 groups
TK = T // 128             # 16 t-chunks
UC = T // 512             # 4 u-chunks of 512
HK = H // 128             # 2 feature blocks
JB0 = {"i": 0, "f": 2, "g": 4, "o": 6}
# window chunking per pass: list of (ws, cw)
CHUNKS = {
    1: [(0, 512), (512, 512), (1024, 512), (1536, 512)],
    2: [(0, 512), (512, 512)],
    3: [(0, 341), (341, 341)],
    4: [(0, 512)],
}

_COMPILED = None


def _build_program():
    import concourse.mybir as mybir
    import concourse.tile as tile
    from concourse import bacc

    f32 = mybir.dt.float32
    bf16 = mybir.dt.bfloat16

    nc = bacc.Bacc("TRN2", target_bir_lowering=False, debug=False)

    f8 = mybir.dt.float8e4
    io = dict(
        x=nc.dram_tensor("x", [BL, T, D], f8, kind="ExternalInput").ap(),
        adjT=nc.dram_tensor("adjT", [T, T], f8, kind="ExternalInput").ap(),
        rs=nc.dram_tensor("rs", [1, T], bf16, kind="ExternalInput").ap(),
        wg0=nc.dram_tensor("wg0", [D, H], bf16, kind="ExternalInput").ap(),
        wg1=nc.dram_tensor("wg1", [D, H], bf16, kind="ExternalInput").ap(),
        bg0=nc.dram_tensor("bg0", [1, H], bf16, kind="ExternalInput").ap(),
        bg1=nc.dram_tensor("bg1", [1, H], bf16, kind="ExternalInput").ap(),
        wihT=nc.dram_tensor("wihT", [MAX_SKIP, H, 4 * H], bf16, kind="ExternalInput").ap(),
        whhT=nc.dram_tensor("whhT", [MAX_SKIP, H, 4 * H], bf16, kind="ExternalInput").ap(),
        biasT=nc.dram_tensor("biasT", [128, MAX_SKIP * 8], f32, kind="ExternalInput").ap(),
        out=nc.dram_tensor("out", [BL, HK, 128, T], bf16, kind="ExternalOutput").ap(),
    )

    with tile.TileContext(nc) as tc:
        _emit(nc, tc, mybir, io)

    nc.compile()
    return nc


class _Ctx:
    pass


def _emit(nc, tc, mybir, io):
    from contextlib import ExitStack

    f32 = mybir.dt.float32
    bf16 = mybir.dt.bfloat16
    AF = mybir.ActivationFunctionType

    c = _Ctx()
    c.nc, c.io, c.f32, c.bf16, c.AF = nc, io, f32, bf16, AF
    c.f8 = mybir.dt.float8e4
    c.DR = mybir.MatmulPerfMode.DoubleRow
    c.ALU = mybir.AluOpType

    with ExitStack() as root:
        # ---- constants / weights ----
        cp = root.enter_context(tc.tile_pool(name="const", bufs=1))
        c.wg0_sb = cp.tile([128, HK * H], bf16, name="wg0_sb")
        c.wg1_sb = cp.tile([128, HK * H], bf16, name="wg1_sb")
        for hk in range(HK):
            nc.scalar.dma_start(out=c.wg0_sb[:, hk * H:(hk + 1) * H],
                                in_=io["wg0"][hk * 128:(hk + 1) * 128, :])
            nc.scalar.dma_start(out=c.wg1_sb[:, hk * H:(hk + 1) * H],
                                in_=io["wg1"][hk * 128:(hk + 1) * 128, :])
        c.bg0_sb = cp.tile([1, H], bf16, name="bg0_sb")
        c.bg1_sb = cp.tile([1, H], bf16, name="bg1_sb")
        c.rs_sb = cp.tile([1, T], bf16, name="rs_sb")
        c.biasT_sb = cp.tile([128, MAX_SKIP * 8], f32, name="biasT_sb")
        nc.scalar.dma_start(out=c.bg0_sb[:], in_=io["bg0"][:])
        nc.scalar.dma_start(out=c.bg1_sb[:], in_=io["bg1"][:])
        nc.scalar.dma_start(out=c.rs_sb[:], in_=io["rs"][:])
        nc.scalar.dma_start(out=c.biasT_sb[:], in_=io["biasT"][:])

        # ---- persistent pools ----
        c.h2t_pool = root.enter_context(tc.tile_pool(name="h2tp", bufs=1))
        c.h1_pool = root.enter_context(tc.tile_pool(name="h1p", bufs=1))
        c.x_pool = root.enter_context(tc.tile_pool(name="xp", bufs=1))
        c.adjt_pool = root.enter_context(tc.tile_pool(name="adjt", bufs=2))
        c.z_pool = root.enter_context(tc.tile_pool(name="zp", bufs=2))
        c.zps = root.enter_context(tc.tile_pool(name="zps", bufs=1, space="PSUM"))
        c.mps = root.enter_context(tc.tile_pool(name="mps", bufs=2, space="PSUM"))
        c.gps = root.enter_context(tc.tile_pool(name="gps", bufs=1, space="PSUM"))
        c.act_pool = root.enter_context(tc.tile_pool(name="actp", bufs=2))
        c.st_pool = root.enter_context(tc.tile_pool(name="stp", bufs=1))
        c.xc_pool = root.enter_context(tc.tile_pool(name="xcp", bufs=2))

        lwp = root.enter_context(tc.tile_pool(name="lw", bufs=1))

        # h2t slabs: 4 tags (2 groups in flight); h1: 2 tags; x: 2x16 tags
        c.h2t = {}   # batch index -> tile (assigned per group)
        c.qidx = 0   # rotating LSTM gate-psum tag

        # group 0 GCN emitted densely
        for blk in _gcn_blocks(c, tc, 0):
            blk()

        # LSTM weights (emitted after group-0 GCN so their DMA doesn't
        # compete with the critical startup x/adjT loads; scalar queue)
        c.wih = {}
        c.whh = {}
        for p in range(1, MAX_SKIP + 1):
            w = lwp.tile([128, HK * 4 * H], bf16, name=f"wih{p}")
            for hk in range(HK):
                nc.scalar.dma_start(out=w[:, hk * 4 * H:(hk + 1) * 4 * H],
                                    in_=io["wihT"][p - 1, hk * 128:(hk + 1) * 128, :])
            c.wih[p] = w
            if p > 1:
                w2 = lwp.tile([128, HK * 4 * H], bf16, name=f"whh{p}")
                for hk in range(HK):
                    nc.scalar.dma_start(out=w2[:, hk * 4 * H:(hk + 1) * 4 * H],
                                        in_=io["whhT"][p - 1, hk * 128:(hk + 1) * 128, :])
                c.whh[p] = w2

        # interleave: LSTM(g) with GCN(g+1); extra filler at pass boundaries
        for g in range(NGRP):
            gcn_next = _gcn_blocks(c, tc, g + 1) if g + 1 < NGRP else []
            gi = 0

            def pump(n):
                nonlocal gi
                for _ in range(n):
                    if gi < len(gcn_next):
                        gcn_next[gi]()
                        gi += 1
            for is_pass_end, pair in _lstm_pairs(c, tc, g):
                pair()
                pump(3 if is_pass_end else 1)
            pump(len(gcn_next))
            # output DMA for this group's batches
            for jj in range(GJ):
                b = g * GJ + jj
                slab = c.h2t[b]
                for hk in range(HK):
                    nc.gpsimd.dma_start(out=io["out"][b, hk],
                                        in_=slab[:, hk * T:(hk + 1) * T])


def _gcn_blocks(c, tc, g):
    """Return list of emission callbacks for group g's GCN (both layers)."""
    if g >= NGRP:
        return []
    nc, io, f32, bf16, AF = c.nc, c.io, c.f32, c.bf16, c.AF
    blocks = []

    KC = TK // 2   # 8 double-row contraction chunks of 256
    xs = {}
    h1 = {}
    adjt = {}
    loaded = set()
    f8 = c.f8

    def load_adjt_k(u4, kc):
        us = u4 * 512
        a = c.adjt_pool.tile([128, 1024], f8, name=f"adjt_{kc}", tag=f"a_{kc}")
        nc.sync.dma_start(
            out=a.rearrange("p (a u) -> p a u", a=2),
            in_=io["adjT"][kc * 256:(kc + 1) * 256, us:us + 512]
            .rearrange("(a p) u -> p a u", a=2))
        adjt[kc] = a

    def load_adjt(u4, phase):
        if (u4, phase) in loaded:
            return
        loaded.add((u4, phase))
        for kc in range(KC):
            load_adjt_k(u4, kc)

    def load_x_k(jj, kc):
        b = g * GJ + jj
        xt = c.x_pool.tile([128, 512], f8, name="xt", tag=f"x_{jj}_{kc}")
        nc.sync.dma_start(
            out=xt.rearrange("p (a d) -> p a d", a=2),
            in_=io["x"][b, kc * 256:(kc + 1) * 256, :]
            .rearrange("(a p) d -> p a d", a=2))
        xs[(jj, kc)] = xt

    def load_first():
        # startup-critical: interleave adjT(u0) / x(jj0) in consumption order
        loaded.add((0, 1))
        for jj in range(GJ):
            b = g * GJ + jj
            h1[jj] = c.h1_pool.tile([128, TK * H], f8, name=f"h1_{jj}",
                                    tag=f"h1_{jj}")
            c.h2t[b] = c.h2t_pool.tile([128, HK * T], bf16, name=f"h2t_{b % 4}",
                                       tag=f"h2t_{b % 4}")
        for kc in range(KC):
            load_adjt_k(0, kc)
            load_x_k(0, kc)
        for kc in range(KC):
            load_x_k(1, kc)
    blocks.append(load_first)

    def ph1_block(u4, jj):
        if jj == 0:
            load_adjt(u4, 1)
        us = u4 * 512
        zp = c.zps.tile([128, 1024], f32, name="zp", tag="zp")
        for kc in range(KC):
            xv = xs[(jj, kc)].rearrange("p (a d) -> p a d", a=2)
            av = adjt[kc].rearrange("p (a u) -> p a u", a=2)
            for dk in range(HK):
                nc.tensor.matmul(zp[:, dk * 512:(dk + 1) * 512],
                                 xv[:, :, dk * 128:(dk + 1) * 128],
                                 av,
                                 start=(kc == 0), stop=(kc == KC - 1),
                                 perf_mode=c.DR)
        z1t = c.z_pool.tile([128, 1024], bf16, name="z1t", tag="z1t")
        nc.vector.tensor_copy(z1t[:], zp[:])
        for ub in range(4):
            ua = us + ub * 128
            hp = c.mps.tile([128, 512], f32, name="hp", tag="mp")
            for dk in range(HK):
                nc.tensor.matmul(hp[:, :H],
                                 z1t[:, dk * 512 + ub * 128: dk * 512 + (ub + 1) * 128],
                                 c.wg0_sb[:, dk * H:(dk + 1) * H],
                                 start=(dk == 0), stop=False)
            nc.tensor.matmul(hp[:, :H], c.rs_sb[0:1, ua:ua + 128], c.bg0_sb[0:1, :],
                             start=False, stop=True)
            # h1 stored as fp8(relu(.)*8); the 1/8 is folded into Wg1
            nc.vector.tensor_scalar(
                h1[jj][:, (u4 * 4 + ub) * H:(u4 * 4 + ub + 1) * H],
                hp[:, :H], 8.0, 0.0, op0=c.ALU.mult, op1=c.ALU.max)

    def ph2_block(u4, jj):
        if jj == 0:
            load_adjt(u4, 2)
        us = u4 * 512
        b = g * GJ + jj
        slab = c.h2t[b]
        zp = c.zps.tile([128, 1024], f32, name="zp2", tag="zp")
        for hk in range(HK):
            for kc in range(KC):
                hv = h1[jj][:, kc * 512:(kc + 1) * 512].rearrange(
                    "p (a h) -> p a h", a=2)
                av = adjt[kc].rearrange("p (a u) -> p a u", a=2)
                nc.tensor.matmul(zp[:, hk * 512:(hk + 1) * 512],
                                 hv[:, :, hk * 128:(hk + 1) * 128],
                                 av,
                                 start=(kc == 0), stop=(kc == KC - 1),
                                 perf_mode=c.DR)
        z2t = c.z_pool.tile([128, 1024], bf16, name="z2t", tag="z2t")
        nc.vector.tensor_copy(z2t[:], zp[:])
        for ho in range(HK):
            hp = c.mps.tile([128, 512], f32, name="hp2", tag="mp")
            for hk in range(HK):
                nc.tensor.matmul(hp[:],
                                 c.wg1_sb[:, hk * H + ho * 128: hk * H + (ho + 1) * 128],
                                 z2t[:, hk * 512:(hk + 1) * 512],
                                 start=(hk == 0), stop=False)
            nc.tensor.matmul(hp[:], c.bg1_sb[0:1, ho * 128:(ho + 1) * 128],
                             c.rs_sb[0:1, us:us + 512], start=False, stop=True)
            nc.vector.tensor_relu(slab[:, ho * T + us: ho * T + us + 512], hp[:])

    for u4 in range(UC):
        for jj in range(GJ):
            blocks.append(lambda u4=u4, jj=jj: ph1_block(u4, jj))
    for u4 in range(UC):
        for jj in range(GJ):
            blocks.append(lambda u4=u4, jj=jj: ph2_block(u4, jj))
    return blocks


def _lstm_pairs(c, tc, g):
    """Yield (is_pass_end, callback), one per unit-pair, passes p=1..4."""
    for p in range(1, MAX_SKIP + 1):
        units = [(jj, ws, cw) for jj in range(GJ) for (ws, cw) in CHUNKS[p]]
        for i in range(0, len(units), 2):
            pair = units[i:i + 2]
            yield (i + 2 >= len(units),
                   lambda p=p, pair=tuple(pair): _emit_pair(c, tc, g, p, pair))


def _emit_pair(c, tc, g, p, pair):
    nc, f32, bf16, AF = c.nc, c.f32, c.bf16, c.AF
    nw = T // p

    # per-unit state dicts
    st = []
    for slot, (jj, ws, cw) in enumerate(pair):
        b = g * GJ + jj
        slab = c.h2t[b]
        view = [slab[:, hk * T: hk * T + nw * p].rearrange("a (w q) -> a w q", q=p)
                for hk in range(HK)]
        u = dict(slot=slot, jj=jj, ws=ws, cw=cw, view=view, slab=slab,
                 c_t=None, h_t=None, xc={})
        st.append(u)

    # prefetch gathers for all steps (gpsimd); pass>1 only
    if p > 1:
        for u in st:
            for t in range(p):
                xc = c.xc_pool.tile([128, 1024], bf16, name="xc",
                                    tag=f"xc{u['slot']}_{t}")
                for hk in range(HK):
                    nc.gpsimd.tensor_copy(
                        xc[:, hk * 512: hk * 512 + u["cw"]],
                        u["view"][hk][:, u["ws"]:u["ws"] + u["cw"], t:t + 1])
                u["xc"][t] = xc

    for t in range(p):
        gates = "igo" if t == 0 else "ifgo"
        for u in st:
            cw, ws, slot = u["cw"], u["ws"], u["slot"]
            n2 = [slice(0, cw), slice(512, 512 + cw)]
            act = {}
            for gn in gates:
                # one gate = one psum quarter (rotating tag)
                q = c.gps.tile([128, 1024], f32, name="q", tag=f"q{c.qidx % 2}")
                c.qidx += 1
                for half in range(2):
                    jb = JB0[gn] + half
                    o = q[:, half * 512: half * 512 + cw]
                    for hk in range(HK):
                        if p == 1:
                            rhs = u["slab"][:, hk * T + ws: hk * T + ws + cw]
                        else:
                            rhs = u["xc"][t][:, hk * 512: hk * 512 + cw]
                        nc.tensor.matmul(
                            o,
                            c.wih[p][:, hk * 4 * H + jb * 128: hk * 4 * H + (jb + 1) * 128],
                            rhs,
                            start=(hk == 0),
                            stop=(t == 0 and hk == HK - 1))
                    if t > 0:
                        for hk in range(HK):
                            nc.tensor.matmul(
                                o,
                                c.whh[p][:, hk * 4 * H + jb * 128: hk * 4 * H + (jb + 1) * 128],
                                u["h_t"][:, hk * 512: hk * 512 + cw],
                                start=False, stop=(hk == HK - 1))
                # activations straight after this gate's matmuls
                fn = AF.Tanh if gn == "g" else AF.Sigmoid
                a = c.act_pool.tile([128, 1024], bf16, name=f"a_{gn}",
                                    tag=f"a{slot}_{gn}")
                act[gn] = a
                for half in range(2):
                    col = (p - 1) * 8 + JB0[gn] + half
                    nc.scalar.activation(
                        a[:, half * 512: half * 512 + cw],
                        q[:, half * 512: half * 512 + cw],
                        fn, bias=c.biasT_sb[:, col:col + 1])

            # cell math (DVE); strided [128, 2, cw] views over the 1024 tiles
            def v2(tile):
                return tile.rearrange("a (h w) -> a h w", h=2)[:, :, :cw]
            if t == 0:
                cn = c.st_pool.tile([128, 1024], f32, name="cn", tag=f"c{slot}")
                nc.vector.tensor_mul(v2(cn), v2(act["i"]), v2(act["g"]))
                u["c_t"] = cn
            else:
                ig = c.st_pool.tile([128, 1024], bf16, name="ig", tag=f"ig{slot}")
                nc.vector.tensor_mul(v2(ig), v2(act["i"]), v2(act["g"]))
                cn = u["c_t"]
                nc.vector.tensor_mul(v2(cn), v2(act["f"]), v2(cn))
                nc.vector.tensor_add(v2(cn), v2(cn), v2(ig))
            # tanh(c) on ACT (single strided instr, no bias)
            tc_t = c.st_pool.tile([128, 1024], bf16, name="tc", tag=f"tc{slot}",
                                  bufs=2)
            nc.scalar.activation(v2(tc_t), v2(u["c_t"]), AF.Tanh)
            if t == p - 1:
                for hk in range(HK):
                    nc.vector.tensor_mul(
                        u["view"][hk][:, ws:ws + cw, p - 1:p],
                        act["o"][:, hk * 512: hk * 512 + cw],
                        tc_t[:, hk * 512: hk * 512 + cw])
            else:
                hn = c.st_pool.tile([128, 1024], bf16, name="hn", tag=f"h{slot}",
                                    bufs=2)
                nc.vector.tensor_mul(v2(hn), v2(act["o"]), v2(tc_t))
                u["h_t"] = hn


SA = 1024.0   # adjT fp8 prescale (folded out of Wg0/Wg1)
SH = 8.0      # h1 fp8 prescale (folded out of Wg1)


def _prep_host(inputs):
    import ml_dtypes
    bf = ml_dtypes.bfloat16
    f8 = ml_dtypes.float8_e4m3fn

    x = np.asarray(inputs["x"], dtype=np.float32)
    adj = np.asarray(inputs["adj"], dtype=np.float32)
    adjT = np.ascontiguousarray(adj.T * SA).astype(f8)
    rs = adj.sum(axis=1, dtype=np.float32).reshape(1, T).astype(bf)
    wg0 = (np.asarray(inputs["Wg0"], dtype=np.float32) / SA).astype(bf)
    wg1 = (np.asarray(inputs["Wg1"], dtype=np.float32) / (SA * SH)).astype(bf)
    bg0 = np.asarray(inputs["bg0"], dtype=np.float32).reshape(1, H).astype(bf)
    bg1 = np.asarray(inputs["bg1"], dtype=np.float32).reshape(1, H).astype(bf)
    wihT = np.ascontiguousarray(
        np.asarray(inputs["Wih"], dtype=np.float32).transpose(0, 2, 1)).astype(bf)
    whhT = np.ascontiguousarray(
        np.asarray(inputs["Whh"], dtype=np.float32).transpose(0, 2, 1)).astype(bf)
    bias = np.asarray(inputs["bih"], dtype=np.float32) + np.asarray(inputs["bhh"], dtype=np.float32)
    biasT = np.ascontiguousarray(
        bias.reshape(MAX_SKIP, 8, 128).transpose(2, 0, 1).reshape(128, MAX_SKIP * 8))
    shared = dict(adjT=adjT, rs=rs, wg0=wg0, wg1=wg1, bg0=bg0, bg1=bg1,
                  wihT=wihT, whhT=whhT, biasT=biasT)
    xb = x.astype(f8)
    in_maps = []
    for cc in range(NCORES):
        m = dict(shared)
        m["x"] = np.ascontiguousarray(xb[cc * BL:(cc + 1) * BL])
        in_maps.append(m)
    return in_maps


def get_compiled():
    global _COMPILED
    if _COMPILED is None:
        _COMPILED = _build_program()
    return _COMPILED


def kernel(**inputs) -> np.ndarray:
    from concourse.bass_utils import run_bass_kernel_spmd

    nc = get_compiled()
    in_maps = _prep_host(inputs)
    res = run_bass_kernel_spmd(nc, in_maps, list(range(NCORES)))
    outs = []
    for cc in range(NCORES):
        o = np.asarray(res.results[cc]["out"]).astype(np.float32)  # [BL, HK, 128, T]
        outs.append(o.transpose(0, 3, 1, 2).reshape(BL, T, D))
    return np.concatenate(outs, axis=0)
